# revision 11
# baseline (speedup 1.0000x reference)
"""BitMLP (BitNet-style MLP) Trainium2 kernel, 8-way data-parallel over tokens.

reference semantics:
  h   = act_quant(rms_norm(x, gamma)) @ w1q.T   (w1q = per-tensor ternary quant)
  out = act_quant(gelu_exact(h)) @ w2q.T

Key facts exploited:
  * act_quant produces n/scale with n an integer in [-127, 127]  -> n is exact in bf16
  * weight quant produces t*inv_w with t ternary in {-1, 0, 1}   -> t is exact in fp8e4
  * mixed-dtype matmul (fp8 stationary x 16-bit moving) runs at full TensorE
    rate and is integer-exact (probed), so scales are applied afterwards.

Sharding (8 cores on one chip):
  * tokens (4*2048 = 8192) split 1024/core; each core computes its tokens' output
  * weight quantization is cooperative: core c quantizes 1/8 of w1 and w2 to
    ternary fp8, the per-tensor mean(|w|) is combined via a scalar AllReduce,
    and the fp8 weights are AllGathered in 128x128 block layout (w1 in 4 chunks
    so MM1 starts on chunk 0; w2's gather is issued last, it's needed last).
  * all x-side work (rms stats, absmax, quantize) is issued BEFORE the
    weight-quant vector work, so the AllReduce's core-sync wait overlaps the
    x pipeline instead of stalling it.
  * h = gelu(...) stays in SBUF as fp16 (no DRAM roundtrip); it is requantized
    in place per token-half; Q2 of both halves is issued before MM2(half0)'s
    consumers so requant overlaps MM2 on the vector/gpsimd engines.
"""

import os
import sys

for _p in ("/root/.axon_site/_ro/trn_rl_repo", "/opt/trn_rl_repo"):
    if os.path.isdir(_p) and _p not in sys.path:
        sys.path.append(_p)

from contextlib import ExitStack

import numpy as np

from concourse import bacc, bass, masks, mybir, tile
from concourse import bass_utils

F32 = mybir.dt.float32
BF16 = mybir.dt.bfloat16
F16 = mybir.dt.float16
F8 = mybir.dt.float8e4
AF = mybir.ActivationFunctionType
OP = mybir.AluOpType
AX = mybir.AxisListType

NCORES = 8
B, S, DIM, HID = 4, 2048, 2048, 8192
NTOK = B * S            # 8192
TOK = NTOK // NCORES    # 1024 tokens per core
KT = DIM // 128         # 16 k-tiles
HB = HID // 128         # 64 hid blocks
DB = DIM // 128         # 16 dim blocks
HBL = HID // NCORES // 128  # 8 hid blocks owned per core
NAG = 4                 # w1 AllGather split into NAG chunks
HBC = HBL // NAG        # hid blocks per chunk per core
CW = HBC * 128          # w1 columns per chunk per core
MAGIC = 12582912.0      # 1.5 * 2**23: (v + MAGIC) - MAGIC == round-half-even(v)
EPS = 1e-6
W_NELEM = float(DIM * HID)

_cache = {}


def _build(n_cores=NCORES):
    nc = bacc.Bacc("TRN2", target_bir_lowering=False, debug=False, num_devices=n_cores)
    xT = nc.dram_tensor("xT", [DIM, TOK], F32, kind="ExternalInput")
    w1s = nc.dram_tensor("w1s", [DIM, HID // n_cores], F32, kind="ExternalInput")
    w2s = nc.dram_tensor("w2s", [HID // n_cores, DIM], F32, kind="ExternalInput")
    gpt = nc.dram_tensor("gpt", [128, KT], F32, kind="ExternalInput")
    outT = nc.dram_tensor("outT", [DIM, TOK], F32, kind="ExternalOutput")
    rg = [list(range(n_cores))]

    with tile.TileContext(nc) as tc, ExitStack() as ctx:
        misc = ctx.enter_context(tc.tile_pool(name="misc", bufs=1))
        ps_mm = ctx.enter_context(tc.tile_pool(name="ps_mm", bufs=2, space="PSUM"))
        ps_tr = ctx.enter_context(tc.tile_pool(name="ps_tr", bufs=2, space="PSUM"))
        ps_ss = ctx.enter_context(tc.tile_pool(name="ps_ss", bufs=1, space="PSUM"))
        dram = ctx.enter_context(tc.tile_pool(name="dram", bufs=1, space="DRAM"))

        ident = misc.tile([128, 128], F32)
        masks.make_identity(nc, ident[:])
        zero_col = misc.tile([128, 1], F32)
        nc.vector.memset(zero_col[:], 0.0)
        ones_bf = misc.tile([128, 1], BF16)
        nc.vector.memset(ones_bf[:], 1.0)
        ones_f = misc.tile([128, 1], F32)
        nc.vector.memset(ones_f[:], 1.0)
        ones_row = misc.tile([1, 128], F32)
        nc.vector.memset(ones_row[:], 1.0)
        gam = misc.tile([128, KT], F32)
        nc.sync.dma_start(gam[:], gpt[:])

        def bcast_row(dst, src_row, n, off=0):
            """dst[128, off:off+n] = broadcast of src_row[1, n] via PE outer product."""
            for o in range(0, n, 512):
                w = min(512, n - o)
                ps = ps_mm.tile([128, 512], F32, tag="mm0")
                nc.tensor.matmul(ps[:, 0:w], ones_row[:], src_row[:, o:o + w],
                                 start=True, stop=True)
                nc.scalar.activation(dst[:, off + o:off + o + w], ps[:, 0:w], AF.Copy, bias=0.0)

        # DRAM scratch
        ar_in = dram.tile([2, 1], F32)
        ar_out = dram.tile([2, 1], F32, addr_space="Shared")
        t1_store = [dram.tile([HBC, 128, KT * 128], F8, name=f"t1s{i}") for i in range(NAG)]
        t1_g = [dram.tile([n_cores, HBC, 128, KT * 128], F8, addr_space="Shared",
                          name=f"t1g{i}") for i in range(NAG)]
        t2_store = dram.tile([DB, 128, HBL * 128], F8)
        t2_g = dram.tile([n_cores, DB, 128, HBL * 128], F8, addr_space="Shared")
        r1_d = dram.tile([8, 128], F32)
        r2_d = dram.tile([8, 128], F32)

        # persistent broadcast-form per-token scale rows etc.
        s2b = misc.tile([128, TOK], F32)
        i2b = misc.tile([128, TOK], F32)
        s1b = misc.tile([128, TOK], F32)
        acc2 = misc.tile([128, TOK], F32)
        m2row = misc.tile([1, TOK], F32)
        invw = misc.tile([1, 2], F32)
        invwb = misc.tile([128, 2], F32)
        swb = misc.tile([128, 2], F32)
        S12 = misc.tile([128, 2], F32)
        S1 = misc.tile([128, KT], F32)
        S2 = misc.tile([128, HBL], F32)

        # nxT is read through MM1; the pool stays on the outer stack (LIFO)
        xq = ctx.enter_context(tc.tile_pool(name="xq", bufs=1))
        nxT = xq.tile([128, KT * TOK], BF16)

        with ExitStack() as pstack:
            wio = pstack.enter_context(tc.tile_pool(name="wio", bufs=2))
            wqo = pstack.enter_context(tc.tile_pool(name="wqo", bufs=2))
            xio = pstack.enter_context(tc.tile_pool(name="xio", bufs=3))
            scr = pstack.enter_context(tc.tile_pool(name="scr", bufs=3))
            xrow = pstack.enter_context(tc.tile_pool(name="xrow", bufs=1))

            # ========= Phase W: per-tensor |w| sums + AllReduce (early) =====
            for kt in range(KT):
                wt = wio.tile([128, HID // n_cores], F32, tag="w1t")
                nc.sync.dma_start(wt[:], w1s[kt * 128:(kt + 1) * 128, :])
                nc.vector.tensor_reduce(S1[:, kt:kt + 1], wt[:], axis=AX.X, op=OP.add,
                                        apply_absolute_value=True)
            for ht in range(HBL):
                wt2 = wio.tile([128, DIM], F32, tag="w2t")
                nc.sync.dma_start(wt2[:], w2s[ht * 128:(ht + 1) * 128, :])
                nc.vector.tensor_reduce(S2[:, ht:ht + 1], wt2[:], axis=AX.X, op=OP.add,
                                        apply_absolute_value=True)
            nc.vector.tensor_reduce(S12[:, 0:1], S1[:], axis=AX.X, op=OP.add)
            nc.vector.tensor_reduce(S12[:, 1:2], S2[:], axis=AX.X, op=OP.add)
            tot_ps = ps_tr.tile([2, 1], F32, tag="tr")
            nc.tensor.matmul(tot_ps[:], S12[:], ones_f[:], start=True, stop=True)
            tot_sb = misc.tile([2, 1], F32)
            nc.vector.tensor_copy(tot_sb[:], tot_ps[:])
            nc.sync.dma_start(ar_in[:], tot_sb[:])
            nc.gpsimd.collective_compute(
                "AllReduce", OP.add, replica_groups=rg, ins=[ar_in[:]], outs=[ar_out[:]])

            # ========= Phase X: rms stats + absmax + quantize (no AR dep) ===
            acc = xrow.tile([128, TOK], F32)
            nc.vector.memset(acc[:], 0.0)
            ss_ps0 = ps_ss.tile([1, 512], F32, tag="ss0")
            ss_ps1 = ps_ss.tile([1, 512], F32, tag="ss1")
            for kt in range(KT):
                xt = xio.tile([128, TOK], F32, tag="xt")
                nc.sync.dma_start(xt[:], xT[kt * 128:(kt + 1) * 128, :])
                x2 = scr.tile([128, TOK], BF16, tag="x2")
                nc.scalar.activation(x2[:], xt[:], AF.Square, bias=zero_col[:])
                nc.tensor.matmul(ss_ps0[:], ones_bf[:], x2[:, 0:512],
                                 start=(kt == 0), stop=(kt == KT - 1))
                nc.tensor.matmul(ss_ps1[:], ones_bf[:], x2[:, 512:1024],
                                 start=(kt == 0), stop=(kt == KT - 1))
                xg = scr.tile([128, TOK], F32, tag="xg")
                nc.vector.tensor_scalar(xg[:], xt[:], gam[:, kt:kt + 1], None, op0=OP.mult)
                xga = scr.tile([128, TOK], F32, tag="xga")
                nc.scalar.activation(xga[:], xg[:], AF.Abs, bias=zero_col[:])
                nc.vector.tensor_tensor(acc[:], acc[:], xga[:], op=OP.max)

            # per-token sum(x^2) -> broadcast form -> rstd
            ss_row = xrow.tile([1, TOK], F32)
            nc.vector.tensor_copy(ss_row[:, 0:512], ss_ps0[:])
            nc.vector.tensor_copy(ss_row[:, 512:1024], ss_ps1[:])
            vb = xrow.tile([128, TOK], F32)
            bcast_row(vb, ss_row, TOK)
            nc.vector.tensor_scalar(vb[:], vb[:], 1.0 / DIM, EPS, op0=OP.mult, op1=OP.add)
            sqb = xrow.tile([128, TOK], F32)
            nc.scalar.activation(sqb[:], vb[:], AF.Sqrt, bias=zero_col[:])
            bscr = xrow.tile([128, TOK], F32)
            rstdb = xrow.tile([128, TOK], F32)
            nc.vector.reciprocal_approx_accurate(rstdb[:], sqb[:], bscr[:])

            # per-token absmax of x*gamma -> m0 -> broadcast
            m0 = xrow.tile([128, 8], F32)
            for c in range(8):
                pt = ps_tr.tile([128, 128], F32, tag="tr")
                nc.tensor.transpose(pt[:], acc[:, c * 128:(c + 1) * 128], ident[:])
                nc.vector.tensor_reduce(m0[:, c:c + 1], pt[:], axis=AX.X, op=OP.max)
            nc.sync.dma_start(r1_d[:].rearrange("c p -> p c"), m0[:])
            m0row = xrow.tile([1, TOK], F32)
            nc.sync.dma_start(m0row[:], r1_d[:].rearrange("c p -> (c p)")[None, :])
            m0b = xrow.tile([128, TOK], F32)
            bcast_row(m0b, m0row, TOK)
            nc.vector.tensor_tensor(m0b[:], m0b[:], rstdb[:], op=OP.mult)
            nc.vector.tensor_scalar(m0b[:], m0b[:], 1e-5, None, op0=OP.max)
            sxb = xrow.tile([128, TOK], F32)
            nc.vector.reciprocal_approx_accurate(sxb[:], m0b[:], bscr[:])
            nc.vector.tensor_scalar(sxb[:], sxb[:], 127.0, None, op0=OP.mult)
            ixb = xrow.tile([128, TOK], F32)
            nc.vector.reciprocal_approx_accurate(ixb[:], sxb[:], bscr[:])
            rsxb = xrow.tile([128, TOK], F32)
            nc.vector.tensor_tensor(rsxb[:], rstdb[:], sxb[:], op=OP.mult)

            # quantize: n_xT = round((x*gamma) * rstd*sx)  (bf16 ints)
            for kt in range(KT):
                xt = xio.tile([128, TOK], F32, tag="xt")
                nc.sync.dma_start(xt[:], xT[kt * 128:(kt + 1) * 128, :])
                t = scr.tile([128, TOK], F32, tag="xg")
                nc.vector.scalar_tensor_tensor(t[:], xt[:], gam[:, kt:kt + 1], rsxb[:],
                                               op0=OP.mult, op1=OP.mult)
                nc.vector.tensor_scalar(nxT[:, kt * TOK:(kt + 1) * TOK], t[:], MAGIC, MAGIC,
                                        op0=OP.add, op1=OP.subtract)

            # ========= scales from the AR result ===========================
            tot2 = misc.tile([1, 2], F32)
            nc.sync.dma_start(tot2[:], ar_out[:].rearrange("a b -> b a"))
            # inv_w = max(mean|w|, 1e-5); scale_w = 1/inv_w
            nc.vector.tensor_scalar(invw[:], tot2[:], 1.0 / W_NELEM, 1e-5, op0=OP.mult, op1=OP.max)
            sw = misc.tile([1, 2], F32)
            nc.vector.reciprocal(sw[:], invw[:])
            ps_b = ps_tr.tile([128, 2], F32, tag="tr")
            nc.tensor.matmul(ps_b[:], ones_row[:], sw[:], start=True, stop=True)
            nc.scalar.activation(swb[:], ps_b[:], AF.Copy, bias=0.0)
            ps_b2 = ps_tr.tile([128, 2], F32, tag="tr")
            nc.tensor.matmul(ps_b2[:], ones_row[:], invw[:], start=True, stop=True)
            nc.scalar.activation(invwb[:], ps_b2[:], AF.Copy, bias=0.0)
            # s1 = (1/sx) * inv_w1 per token, broadcast form
            nc.vector.tensor_scalar(s1b[:], ixb[:], invwb[:, 0:1], None, op0=OP.mult)

            # ========= Phase WQ: ternary fp8 quantize + chunked AllGather ===
            # w*sw -> clip[-1,1] -> round(: two DVE passes + one ScalarE pass
            # (Copy with bias=-MAGIC), fp8 out). clip-then-round == ternary.
            # high_priority pins the w1 quant+gather chain ahead of the w2 one
            # in the scheduler so the 4 w1 chunk AllGathers (which gate MM1)
            # are queued on the collective stream before w2's big gather.
            with tc.high_priority():
                for ci in range(NAG):
                    for kt in range(KT):
                        wt = wio.tile([128, CW], F32, tag="w1q")
                        nc.sync.dma_start(wt[:], w1s[kt * 128:(kt + 1) * 128,
                                                     ci * CW:(ci + 1) * CW])
                        nc.vector.tensor_scalar(wt[:], wt[:], swb[:, 0:1], -1.0, op0=OP.mult, op1=OP.max)
                        nc.vector.tensor_scalar(wt[:], wt[:], 1.0, MAGIC, op0=OP.min, op1=OP.add)
                        q = wqo.tile([128, CW], F8, tag="wqq")
                        nc.scalar.activation(q[:], wt[:], AF.Copy, bias=-MAGIC)
                        nc.gpsimd.dma_start(
                            t1_store[ci][:, :, kt * 128:(kt + 1) * 128].rearrange(
                                "b k j -> k b j"),
                            q[:].rearrange("k (b j) -> k b j", b=HBC))
                    nc.gpsimd.collective_compute(
                        "AllGather", OP.bypass, replica_groups=rg,
                        ins=[t1_store[ci][:]], outs=[t1_g[ci][:]])
            # w2 quant reuses the SAME pool tags as the w1 chunks: the slot
            # recycling serializes it behind all w1 quant work, which keeps
            # its AllGather behind the 4 w1 chunk AllGathers on the
            # collective stream (MM1 needs those first; w2 is needed last).
            for ht in range(HBL * 2):
                wt2 = wio.tile([128, DIM // 2], F32, tag="w1q")
                nc.sync.dma_start(wt2[:], w2s[ht // 2 * 128:(ht // 2 + 1) * 128,
                                              (ht % 2) * (DIM // 2):(ht % 2 + 1) * (DIM // 2)])
                nc.vector.tensor_scalar(wt2[:], wt2[:], swb[:, 1:2], -1.0, op0=OP.mult, op1=OP.max)
                nc.vector.tensor_scalar(wt2[:], wt2[:], 1.0, MAGIC, op0=OP.min, op1=OP.add)
                q2 = wqo.tile([128, DIM // 2], F8, tag="wqq")
                nc.scalar.activation(q2[:], wt2[:], AF.Copy, bias=-MAGIC)
                nc.gpsimd.dma_start(
                    t2_store[(ht % 2) * (DB // 2):(ht % 2 + 1) * (DB // 2), :,
                             ht // 2 * 128:(ht // 2 + 1) * 128].rearrange("d k j -> k d j"),
                    q2[:].rearrange("k (d j) -> k d j", d=DB // 2))
            nc.gpsimd.collective_compute(
                "AllGather", OP.bypass, replica_groups=rg, ins=[t2_store[:]], outs=[t2_g[:]])

        # ========= MM1: h = gelu((n_x @ t1) * s1) -> fp16 in SBUF ===========
        # hbig allocated after the prologue pools close so prologue + hbig
        # never coexist in SBUF; per token-half tiles so MM2(h0) reads and
        # Q2(h1) writes don't share a tile.
        hpool = ctx.enter_context(tc.tile_pool(name="hpool", bufs=1))
        hbig = [hpool.tile([128, HB * 512], F16, tag=f"hbig{t}", name=f"hbig{t}")
                for t in range(2)]

        with ExitStack() as mctx:
            w1st = mctx.enter_context(tc.tile_pool(name="w1st", bufs=3))
            scr1 = mctx.enter_context(tc.tile_pool(name="scr1", bufs=3))

            nc.vector.memset(acc2[:], 0.0)
            for ci in range(NAG):
                for r in range(n_cores):
                    for bi in range(HBC):
                        hb = r * HBL + ci * HBC + bi
                        wb = w1st.tile([128, KT * 128], F8, tag="wb")
                        nc.sync.dma_start(wb[:], t1_g[ci][r, bi])
                        ps0 = ps_mm.tile([128, 512], F32, tag="mm0")
                        ps1 = ps_mm.tile([128, 512], F32, tag="mm1")
                        for kt in range(KT):
                            st, sp = (kt == 0), (kt == KT - 1)
                            nc.tensor.matmul(ps0[:], wb[:, kt * 128:(kt + 1) * 128],
                                             nxT[:, kt * TOK:kt * TOK + 512], start=st, stop=sp)
                            nc.tensor.matmul(ps1[:], wb[:, kt * 128:(kt + 1) * 128],
                                             nxT[:, kt * TOK + 512:kt * TOK + 1024], start=st, stop=sp)
                        for th, ps in ((0, ps0), (1, ps1)):
                            sl = slice(th * 512, th * 512 + 512)
                            hs = scr1.tile([128, 512], F32, tag="hs")
                            nc.vector.tensor_tensor(hs[:], ps[:], s1b[:, sl], op=OP.mult)
                            gd = hbig[th][:, hb * 512:(hb + 1) * 512]
                            nc.scalar.activation(gd, hs[:], AF.Gelu, bias=zero_col[:])
                            ga = scr1.tile([128, 512], F32, tag="ga")
                            nc.scalar.activation(ga[:], gd, AF.Abs, bias=zero_col[:])
                            nc.vector.tensor_tensor(acc2[:, sl], acc2[:, sl], ga[:], op=OP.max)

        # ========= Q2 + MM2 ================================================
        with ExitStack() as hctx:
            w2st = hctx.enter_context(tc.tile_pool(name="w2st", bufs=2))
            scr2 = hctx.enter_context(tc.tile_pool(name="scr2", bufs=3))
            hio = hctx.enter_context(tc.tile_pool(name="hio", bufs=2))

            # scale2 rows from acc2
            m2 = misc.tile([128, 8], F32)
            for c in range(8):
                pt = ps_tr.tile([128, 128], F32, tag="tr")
                nc.tensor.transpose(pt[:], acc2[:, c * 128:(c + 1) * 128], ident[:])
                nc.vector.tensor_reduce(m2[:, c:c + 1], pt[:], axis=AX.X, op=OP.max)
            nc.sync.dma_start(r2_d[:].rearrange("c p -> p c"), m2[:])
            nc.sync.dma_start(m2row[:], r2_d[:].rearrange("c p -> (c p)")[None, :])
            bcast_row(s2b, m2row, TOK)
            nc.vector.tensor_scalar(s2b[:], s2b[:], 1e-5, None, op0=OP.max)
            # s1b is dead after MM1 -> reuse it as the reciprocal scratch
            nc.vector.reciprocal_approx_accurate(i2b[:], s2b[:], s1b[:])
            nc.vector.tensor_scalar(s2b[:], i2b[:], 127.0, None, op0=OP.mult)
            nc.vector.reciprocal_approx_accurate(i2b[:], s2b[:], s1b[:])
            nc.vector.tensor_scalar(i2b[:], i2b[:], invwb[:, 1:2], None, op0=OP.mult)

            # requantize h in place: hq = round(g * s2); mult on DVE, round on
            # GpSimd so the passes pipeline. Both halves issued before MM2's
            # consumers so Q2(h1) overlaps MM2(h0) matmuls.
            for th in range(2):
                to = th * 512
                for hb in range(HB):
                    gsl = hbig[th][:, hb * 512:(hb + 1) * 512]
                    tq = scr2.tile([128, 512], F32, tag="tq")
                    nc.vector.tensor_tensor(tq[:], gsl, s2b[:, to:to + 512], op=OP.mult)
                    nc.gpsimd.tensor_scalar(gsl, tq[:], MAGIC, MAGIC, op0=OP.add, op1=OP.subtract)

            nr2 = n_cores // 2
            HK = HB // 2
            for th in range(2):
                to = th * 512
                for d in range(DB):
                    wA = w2st.tile([128, HK * 128], F8, tag="wA")
                    nc.sync.dma_start(
                        wA[:].rearrange("k (r f) -> k r f", r=nr2),
                        t2_g[0:nr2, d].rearrange("r k f -> k r f"))
                    wB = w2st.tile([128, HK * 128], F8, tag="wB")
                    nc.sync.dma_start(
                        wB[:].rearrange("k (r f) -> k r f", r=nr2),
                        t2_g[nr2:n_cores, d].rearrange("r k f -> k r f"))
                    ps = ps_mm.tile([128, 512], F32, tag=f"mm{th}")
                    for kg in range(HB):
                        st, sp = (kg == 0), (kg == HB - 1)
                        w_ = wA if kg < HK else wB
                        ko = (kg % HK) * 128
                        nc.tensor.matmul(ps[:], w_[:, ko:ko + 128],
                                         hbig[th][:, kg * 512:(kg + 1) * 512], start=st, stop=sp)
                    ot = hio.tile([128, 512], F32, tag="ot")
                    nc.vector.tensor_tensor(ot[:], ps[:], i2b[:, to:to + 512], op=OP.mult)
                    nc.sync.dma_start(outT[d * 128:(d + 1) * 128, to:to + 512], ot[:])

    nc.compile()
    return nc


def _get_nc():
    if "nc" not in _cache:
        _cache["nc"] = _build()
    return _cache["nc"]


def _prep_inputs(x, w1, w2, gamma):
    x2d = np.ascontiguousarray(np.asarray(x, dtype=np.float32).reshape(NTOK, DIM))
    w1 = np.asarray(w1, dtype=np.float32)
    w2 = np.asarray(w2, dtype=np.float32)
    gamma = np.asarray(gamma, dtype=np.float32)
    w1T = np.ascontiguousarray(w1.T)          # [DIM, HID]
    w2T = np.ascontiguousarray(w2.T)          # [HID, DIM]
    gpt = np.ascontiguousarray(gamma.reshape(KT, 128).T)
    hs = HID // NCORES
    in_maps = []
    for c in range(NCORES):
        in_maps.append({
            "xT": np.ascontiguousarray(x2d[c * TOK:(c + 1) * TOK, :].T),
            "w1s": np.ascontiguousarray(w1T[:, c * hs:(c + 1) * hs]),
            "w2s": np.ascontiguousarray(w2T[c * hs:(c + 1) * hs, :]),
            "gpt": gpt,
        })
    return in_maps


def _run(in_maps, trace=False, **kw):
    nc = _get_nc()
    return bass_utils.run_bass_kernel_spmd(
        nc, in_maps, core_ids=list(range(NCORES)), trace=trace, **kw)


def kernel(x, w1, w2, gamma):
    in_maps = _prep_inputs(x, w1, w2, gamma)
    res = _run(in_maps, trace=False)
    out = np.empty((NTOK, DIM), dtype=np.float32)
    for c in range(NCORES):
        out[c * TOK:(c + 1) * TOK, :] = res.results[c]["outT"].T
    return out.reshape(B, S, DIM)


# revision 22
# speedup vs baseline: 1.3990x; 1.3990x over previous
"""BitMLP (BitNet-style MLP) Trainium2 kernel, 8-way data-parallel over tokens.

reference semantics:
  h   = act_quant(rms_norm(x, gamma)) @ w1q.T   (w1q = per-tensor ternary quant)
  out = act_quant(gelu_exact(h)) @ w2q.T

Key facts exploited:
  * act_quant produces n/scale with n an integer in [-127, 127]  -> n is exact in bf16
  * weight quant produces t*inv_w with t ternary in {-1, 0, 1}   -> t is exact in fp8e4
  * mixed-dtype matmul (fp8 stationary x 16-bit moving) runs at full TensorE
    rate and is integer-exact (probed), so scales are applied afterwards.

Sharding (8 cores on one chip):
  * tokens (4*2048 = 8192) split 1024/core; each core computes its tokens' output
  * weight quantization is cooperative: core c quantizes 1/8 of w1 and w2 to
    ternary fp8, the per-tensor mean(|w|) is combined via a scalar AllReduce,
    and the fp8 weights are AllGathered in 128x128 block layout (w1 in 4 chunks
    so MM1 starts on chunk 0; w2's gather is issued last, it's needed last).
  * all x-side work (rms stats, absmax, quantize) is issued BEFORE the
    weight-quant vector work, so the AllReduce's core-sync wait overlaps the
    x pipeline instead of stalling it.
  * h = gelu(...) stays in SBUF as bf16 (no DRAM roundtrip; bf16 is the
    full-speed DVE dtype - fp16 hits a ~10x slow path). It is requantized in
    place per token-half in 8-block sub-tiles so MM2(half0) chases Q2(half0)
    at sub-tile granularity and Q2(half1) overlaps MM2(half0).
"""

import os
import sys

for _p in ("/root/.axon_site/_ro/trn_rl_repo", "/opt/trn_rl_repo"):
    if os.path.isdir(_p) and _p not in sys.path:
        sys.path.append(_p)

from contextlib import ExitStack

import numpy as np

from concourse import bacc, bass, masks, mybir, tile
from concourse import bass_utils

F32 = mybir.dt.float32
BF16 = mybir.dt.bfloat16
F16 = mybir.dt.float16
F8 = mybir.dt.float8e4
AF = mybir.ActivationFunctionType
OP = mybir.AluOpType
AX = mybir.AxisListType

NCORES = 8
B, S, DIM, HID = 4, 2048, 2048, 8192
NTOK = B * S            # 8192
TOK = NTOK // NCORES    # 1024 tokens per core
KT = DIM // 128         # 16 k-tiles
HB = HID // 128         # 64 hid blocks
DB = DIM // 128         # 16 dim blocks
HBL = HID // NCORES // 128  # 8 hid blocks owned per core
NAG = 4                 # w1 AllGather split into NAG chunks
HBC = HBL // NAG        # hid blocks per chunk per core
CW = HBC * 128          # w1 columns per chunk per core
MAGIC = 12582912.0      # 1.5 * 2**23: (v + MAGIC) - MAGIC == round-half-even(v)
EPS = 1e-6
W_NELEM = float(DIM * HID)

_cache = {}


def _build(n_cores=NCORES):
    nc = bacc.Bacc("TRN2", target_bir_lowering=False, debug=False, num_devices=n_cores)
    xT = nc.dram_tensor("xT", [DIM, TOK], F32, kind="ExternalInput")
    w1s = nc.dram_tensor("w1s", [DIM, HID // n_cores], F32, kind="ExternalInput")
    w2s = nc.dram_tensor("w2s", [HID // n_cores, DIM], F32, kind="ExternalInput")
    gpt = nc.dram_tensor("gpt", [128, KT], F32, kind="ExternalInput")
    outT = nc.dram_tensor("outT", [DIM, TOK], F32, kind="ExternalOutput")
    rg = [list(range(n_cores))]

    with tile.TileContext(nc) as tc, ExitStack() as ctx:
        misc = ctx.enter_context(tc.tile_pool(name="misc", bufs=1))
        ps_mm = ctx.enter_context(tc.tile_pool(name="ps_mm", bufs=2, space="PSUM"))
        ps_tr = ctx.enter_context(tc.tile_pool(name="ps_tr", bufs=2, space="PSUM"))
        ps_ss = ctx.enter_context(tc.tile_pool(name="ps_ss", bufs=1, space="PSUM"))
        dram = ctx.enter_context(tc.tile_pool(name="dram", bufs=1, space="DRAM"))

        ident = misc.tile([128, 128], F32)
        masks.make_identity(nc, ident[:])
        zero_col = misc.tile([128, 1], F32)
        nc.vector.memset(zero_col[:], 0.0)
        ones_bf = misc.tile([128, 1], BF16)
        nc.vector.memset(ones_bf[:], 1.0)
        ones_f = misc.tile([128, 1], F32)
        nc.vector.memset(ones_f[:], 1.0)
        ones_row = misc.tile([1, 128], F32)
        nc.vector.memset(ones_row[:], 1.0)
        gam = misc.tile([128, KT], F32)
        nc.sync.dma_start(gam[:], gpt[:])

        def bcast_row(dst, src_row, n, off=0):
            """dst[128, off:off+n] = broadcast of src_row[1, n] via PE outer product."""
            for o in range(0, n, 512):
                w = min(512, n - o)
                ps = ps_mm.tile([128, 512], F32, tag="mm0")
                nc.tensor.matmul(ps[:, 0:w], ones_row[:], src_row[:, o:o + w],
                                 start=True, stop=True)
                nc.scalar.activation(dst[:, off + o:off + o + w], ps[:, 0:w], AF.Copy, bias=0.0)

        # DRAM scratch
        ar_in = dram.tile([2, 1], F32)
        ar_out = dram.tile([2, 1], F32, addr_space="Shared")
        t1_store = [dram.tile([HBC, 128, KT * 128], F8, name=f"t1s{i}") for i in range(NAG)]
        t1_g = [dram.tile([n_cores, HBC, 128, KT * 128], F8, addr_space="Shared",
                          name=f"t1g{i}") for i in range(NAG)]
        t2_store = dram.tile([DB, 128, HBL * 128], F8)
        t2_g = dram.tile([n_cores, DB, 128, HBL * 128], F8, addr_space="Shared")
        r1_d = dram.tile([8, 128], F32)
        r2_d = dram.tile([8, 128], F32)

        # persistent broadcast-form per-token scale rows etc.
        s2b = misc.tile([128, TOK], F32)
        i2b = misc.tile([128, TOK], F32)
        s1b = misc.tile([128, TOK], F32)
        acc2 = misc.tile([128, TOK], F32)
        m2row = misc.tile([1, TOK], F32)
        invw = misc.tile([1, 2], F32)
        invwb = misc.tile([128, 2], F32)
        swb = misc.tile([128, 2], F32)
        S12 = misc.tile([128, 2], F32)
        S1 = misc.tile([128, KT], F32)
        S2 = misc.tile([128, HBL], F32)

        # nxT is read through MM1; the pool stays on the outer stack (LIFO)
        xq = ctx.enter_context(tc.tile_pool(name="xq", bufs=1))
        nxT = xq.tile([128, KT * TOK], BF16)

        with ExitStack() as pstack:
            wio = pstack.enter_context(tc.tile_pool(name="wio", bufs=2))
            wqo = pstack.enter_context(tc.tile_pool(name="wqo", bufs=2))
            xio = pstack.enter_context(tc.tile_pool(name="xio", bufs=3))
            scr = pstack.enter_context(tc.tile_pool(name="scr", bufs=3))
            xrow = pstack.enter_context(tc.tile_pool(name="xrow", bufs=1))

            # ========= Phase W: per-tensor |w| sums + AllReduce (early) =====
            for kt in range(KT):
                wt = wio.tile([128, HID // n_cores], F32, tag="w1t")
                nc.sync.dma_start(wt[:], w1s[kt * 128:(kt + 1) * 128, :])
                nc.vector.tensor_reduce(S1[:, kt:kt + 1], wt[:], axis=AX.X, op=OP.add,
                                        apply_absolute_value=True)
            for ht in range(HBL):
                wt2 = wio.tile([128, DIM], F32, tag="w2t")
                nc.sync.dma_start(wt2[:], w2s[ht * 128:(ht + 1) * 128, :])
                nc.vector.tensor_reduce(S2[:, ht:ht + 1], wt2[:], axis=AX.X, op=OP.add,
                                        apply_absolute_value=True)
            nc.vector.tensor_reduce(S12[:, 0:1], S1[:], axis=AX.X, op=OP.add)
            nc.vector.tensor_reduce(S12[:, 1:2], S2[:], axis=AX.X, op=OP.add)
            tot_ps = ps_tr.tile([2, 1], F32, tag="tr")
            nc.tensor.matmul(tot_ps[:], S12[:], ones_f[:], start=True, stop=True)
            tot_sb = misc.tile([2, 1], F32)
            nc.vector.tensor_copy(tot_sb[:], tot_ps[:])
            nc.sync.dma_start(ar_in[:], tot_sb[:])
            nc.gpsimd.collective_compute(
                "AllReduce", OP.add, replica_groups=rg, ins=[ar_in[:]], outs=[ar_out[:]])

            # ========= Phase X: rms stats + absmax + quantize (no AR dep) ===
            acc = xrow.tile([128, TOK], F32)
            nc.vector.memset(acc[:], 0.0)
            ss_ps0 = ps_ss.tile([1, 512], F32, tag="ss0")
            ss_ps1 = ps_ss.tile([1, 512], F32, tag="ss1")
            for kt in range(KT):
                xt = xio.tile([128, TOK], F32, tag="xt")
                nc.sync.dma_start(xt[:], xT[kt * 128:(kt + 1) * 128, :])
                x2 = scr.tile([128, TOK], BF16, tag="x2")
                nc.scalar.activation(x2[:], xt[:], AF.Square, bias=zero_col[:])
                nc.tensor.matmul(ss_ps0[:], ones_bf[:], x2[:, 0:512],
                                 start=(kt == 0), stop=(kt == KT - 1))
                nc.tensor.matmul(ss_ps1[:], ones_bf[:], x2[:, 512:1024],
                                 start=(kt == 0), stop=(kt == KT - 1))
                xg = scr.tile([128, TOK], F32, tag="xg")
                nc.vector.tensor_scalar(xg[:], xt[:], gam[:, kt:kt + 1], None, op0=OP.mult)
                xga = scr.tile([128, TOK], F32, tag="xga")
                nc.scalar.activation(xga[:], xg[:], AF.Abs, bias=zero_col[:])
                nc.vector.tensor_tensor(acc[:], acc[:], xga[:], op=OP.max)

            # per-token sum(x^2) -> broadcast form -> rstd
            ss_row = xrow.tile([1, TOK], F32)
            nc.vector.tensor_copy(ss_row[:, 0:512], ss_ps0[:])
            nc.vector.tensor_copy(ss_row[:, 512:1024], ss_ps1[:])
            vb = xrow.tile([128, TOK], F32)
            bcast_row(vb, ss_row, TOK)
            nc.vector.tensor_scalar(vb[:], vb[:], 1.0 / DIM, EPS, op0=OP.mult, op1=OP.add)
            sqb = xrow.tile([128, TOK], F32)
            nc.scalar.activation(sqb[:], vb[:], AF.Sqrt, bias=zero_col[:])
            bscr = xrow.tile([128, TOK], F32)
            rstdb = xrow.tile([128, TOK], F32)
            nc.vector.reciprocal_approx_accurate(rstdb[:], sqb[:], bscr[:])

            # per-token absmax of x*gamma -> m0 -> broadcast
            m0 = xrow.tile([128, 8], F32)
            for c in range(8):
                pt = ps_tr.tile([128, 128], F32, tag="tr")
                nc.tensor.transpose(pt[:], acc[:, c * 128:(c + 1) * 128], ident[:])
                nc.vector.tensor_reduce(m0[:, c:c + 1], pt[:], axis=AX.X, op=OP.max)
            nc.sync.dma_start(r1_d[:].rearrange("c p -> p c"), m0[:])
            m0row = xrow.tile([1, TOK], F32)
            nc.sync.dma_start(m0row[:], r1_d[:].rearrange("c p -> (c p)")[None, :])
            m0b = xrow.tile([128, TOK], F32)
            bcast_row(m0b, m0row, TOK)
            nc.vector.tensor_tensor(m0b[:], m0b[:], rstdb[:], op=OP.mult)
            nc.vector.tensor_scalar(m0b[:], m0b[:], 1e-5, None, op0=OP.max)
            sxb = xrow.tile([128, TOK], F32)
            nc.vector.reciprocal_approx_accurate(sxb[:], m0b[:], bscr[:])
            nc.vector.tensor_scalar(sxb[:], sxb[:], 127.0, None, op0=OP.mult)
            ixb = xrow.tile([128, TOK], F32)
            nc.vector.reciprocal_approx_accurate(ixb[:], sxb[:], bscr[:])
            rsxb = xrow.tile([128, TOK], F32)
            nc.vector.tensor_tensor(rsxb[:], rstdb[:], sxb[:], op=OP.mult)

            # ========= scales from the AR result ===========================
            # (loaded on the gpsimd queue so its AR-completion wait does not
            # head-of-line-block the sync DMA queue)
            tot2 = misc.tile([1, 2], F32)
            nc.gpsimd.dma_start(tot2[:], ar_out[:].rearrange("a b -> b a"))
            # inv_w = max(mean|w|, 1e-5); scale_w = 1/inv_w
            nc.vector.tensor_scalar(invw[:], tot2[:], 1.0 / W_NELEM, 1e-5, op0=OP.mult, op1=OP.max)
            sw = misc.tile([1, 2], F32)
            nc.vector.reciprocal(sw[:], invw[:])
            ps_b = ps_tr.tile([128, 2], F32, tag="tr")
            nc.tensor.matmul(ps_b[:], ones_row[:], sw[:], start=True, stop=True)
            nc.scalar.activation(swb[:], ps_b[:], AF.Copy, bias=0.0)
            ps_b2 = ps_tr.tile([128, 2], F32, tag="tr")
            nc.tensor.matmul(ps_b2[:], ones_row[:], invw[:], start=True, stop=True)
            nc.scalar.activation(invwb[:], ps_b2[:], AF.Copy, bias=0.0)
            # s1 = (1/sx) * inv_w1 per token, broadcast form
            nc.vector.tensor_scalar(s1b[:], ixb[:], invwb[:, 0:1], None, op0=OP.mult)

            # ========= Phase WQ: ternary fp8 quantize + chunked AllGather ===
            # w*sw -> clip[-1,1] -> round(: two DVE passes + one ScalarE pass
            # (Copy with bias=-MAGIC), fp8 out). clip-then-round == ternary.
            # high_priority pins the w1 quant+gather chain ahead of the w2 one
            # in the scheduler so the 4 w1 chunk AllGathers (which gate MM1)
            # are queued on the collective stream before w2's big gather.
            # Scatter stores ride the scalar queue (right after the pass-3
            # quant producing them); the gpsimd queue stays collectives-only
            # so a pending AllGather never stalls the next chunk's scatters.
            with tc.high_priority():
                for ci in range(NAG):
                    for kt in range(KT):
                        wt = wio.tile([128, CW], F32, tag="w1q")
                        nc.sync.dma_start(wt[:], w1s[kt * 128:(kt + 1) * 128,
                                                     ci * CW:(ci + 1) * CW])
                        nc.vector.tensor_scalar(wt[:], wt[:], swb[:, 0:1], -1.0, op0=OP.mult, op1=OP.max)
                        nc.vector.tensor_scalar(wt[:], wt[:], 1.0, MAGIC, op0=OP.min, op1=OP.add)
                        q = wqo.tile([128, CW], F8, tag="wqq")
                        nc.scalar.activation(q[:], wt[:], AF.Copy, bias=-MAGIC)
                        nc.scalar.dma_start(
                            t1_store[ci][:, :, kt * 128:(kt + 1) * 128].rearrange(
                                "b k j -> k b j"),
                            q[:].rearrange("k (b j) -> k b j", b=HBC))
                    nc.gpsimd.collective_compute(
                        "AllGather", OP.bypass, replica_groups=rg,
                        ins=[t1_store[ci][:]], outs=[t1_g[ci][:]])

            # quantize x: n_xT = round((x*gamma) * rstd*sx)  (bf16 ints);
            # issued after the w1 chunks so the DVE prioritizes unblocking
            # the gathers, but before the (late-needed) w2 quant.
            for kt in range(KT):
                xt = xio.tile([128, TOK], F32, tag="xt")
                nc.sync.dma_start(xt[:], xT[kt * 128:(kt + 1) * 128, :])
                t = scr.tile([128, TOK], F32, tag="xg")
                nc.vector.scalar_tensor_tensor(t[:], xt[:], gam[:, kt:kt + 1], rsxb[:],
                                               op0=OP.mult, op1=OP.mult)
                nc.vector.tensor_scalar(nxT[:, kt * TOK:(kt + 1) * TOK], t[:], MAGIC, MAGIC,
                                        op0=OP.add, op1=OP.subtract)
            # w2 quant reuses the SAME pool tags as the w1 chunks: the slot
            # recycling serializes it behind all w1 quant work, which keeps
            # its AllGather behind the 4 w1 chunk AllGathers on the
            # collective stream (MM1 needs those first; w2 is needed last).
            for ht in range(HBL * 2):
                wt2 = wio.tile([128, DIM // 2], F32, tag="w1q")
                nc.sync.dma_start(wt2[:], w2s[ht // 2 * 128:(ht // 2 + 1) * 128,
                                              (ht % 2) * (DIM // 2):(ht % 2 + 1) * (DIM // 2)])
                nc.vector.tensor_scalar(wt2[:], wt2[:], swb[:, 1:2], -1.0, op0=OP.mult, op1=OP.max)
                nc.vector.tensor_scalar(wt2[:], wt2[:], 1.0, MAGIC, op0=OP.min, op1=OP.add)
                q2 = wqo.tile([128, DIM // 2], F8, tag="wqq")
                nc.scalar.activation(q2[:], wt2[:], AF.Copy, bias=-MAGIC)
                nc.scalar.dma_start(
                    t2_store[(ht % 2) * (DB // 2):(ht % 2 + 1) * (DB // 2), :,
                             ht // 2 * 128:(ht // 2 + 1) * 128].rearrange("d k j -> k d j"),
                    q2[:].rearrange("k (d j) -> k d j", d=DB // 2))
            nc.gpsimd.collective_compute(
                "AllGather", OP.bypass, replica_groups=rg, ins=[t2_store[:]], outs=[t2_g[:]])

        # ========= MM1: h = gelu((n_x @ t1) * s1) -> bf16 in SBUF ===========
        # hbig allocated after the prologue pools close so prologue + hbig
        # never coexist in SBUF. 8 sub-tiles of 8 hid-blocks per token-half:
        # RAW tracking is tile-granular, so sub-tiles let MM2(h0) start after
        # Q2 finishes a sub-tile rather than the whole half.
        NSUB = 8
        SUBB = HB // NSUB  # hid blocks per sub-tile
        hpool = ctx.enter_context(tc.tile_pool(name="hpool", bufs=1))
        hsub = [[hpool.tile([128, SUBB * 512], BF16, tag=f"hs{t}_{j}", name=f"hs{t}_{j}")
                 for j in range(NSUB)] for t in range(2)]

        def hslice(th, hb):
            return hsub[th][hb // SUBB][:, (hb % SUBB) * 512:(hb % SUBB + 1) * 512]

        with ExitStack() as mctx:
            w1st = mctx.enter_context(tc.tile_pool(name="w1st", bufs=3))
            scr1 = mctx.enter_context(tc.tile_pool(name="scr1", bufs=3))

            nc.vector.memset(acc2[:], 0.0)
            for ci in range(NAG):
                for r in range(n_cores):
                    for bi in range(HBC):
                        hb = r * HBL + ci * HBC + bi
                        wb = w1st.tile([128, KT * 128], F8, tag="wb")
                        nc.sync.dma_start(wb[:], t1_g[ci][r, bi])
                        ps0 = ps_mm.tile([128, 512], F32, tag="mm0")
                        ps1 = ps_mm.tile([128, 512], F32, tag="mm1")
                        for kt in range(KT):
                            st, sp = (kt == 0), (kt == KT - 1)
                            nc.tensor.matmul(ps0[:], wb[:, kt * 128:(kt + 1) * 128],
                                             nxT[:, kt * TOK:kt * TOK + 512], start=st, stop=sp)
                            nc.tensor.matmul(ps1[:], wb[:, kt * 128:(kt + 1) * 128],
                                             nxT[:, kt * TOK + 512:kt * TOK + 1024], start=st, stop=sp)
                        for th, ps in ((0, ps0), (1, ps1)):
                            sl = slice(th * 512, th * 512 + 512)
                            hs = scr1.tile([128, 512], F32, tag="hs")
                            nc.vector.tensor_tensor(hs[:], ps[:], s1b[:, sl], op=OP.mult)
                            gd = hslice(th, hb)
                            nc.scalar.activation(gd, hs[:], AF.Gelu, bias=zero_col[:])
                            ga = scr1.tile([128, 512], F32, tag="ga")
                            nc.scalar.activation(ga[:], gd, AF.Abs, bias=zero_col[:])
                            nc.vector.tensor_tensor(acc2[:, sl], acc2[:, sl], ga[:], op=OP.max)

        # ========= Q2 + MM2 ================================================
        with ExitStack() as hctx:
            w2st = hctx.enter_context(tc.tile_pool(name="w2st", bufs=2))
            w2stB = hctx.enter_context(tc.tile_pool(name="w2stB", bufs=1))
            scr2 = hctx.enter_context(tc.tile_pool(name="scr2", bufs=2))
            hio = hctx.enter_context(tc.tile_pool(name="hio", bufs=2))

            # scale2 rows from acc2
            m2 = misc.tile([128, 8], F32)
            for c in range(8):
                pt = ps_tr.tile([128, 128], F32, tag="tr")
                nc.tensor.transpose(pt[:], acc2[:, c * 128:(c + 1) * 128], ident[:])
                nc.vector.tensor_reduce(m2[:, c:c + 1], pt[:], axis=AX.X, op=OP.max)
            nc.sync.dma_start(r2_d[:].rearrange("c p -> p c"), m2[:])
            nc.sync.dma_start(m2row[:], r2_d[:].rearrange("c p -> (c p)")[None, :])
            bcast_row(s2b, m2row, TOK)
            nc.vector.tensor_scalar(s2b[:], s2b[:], 1e-5, None, op0=OP.max)
            # s1b is dead after MM1 -> reuse it as the reciprocal scratch
            nc.vector.reciprocal_approx_accurate(i2b[:], s2b[:], s1b[:])
            nc.vector.tensor_scalar(s2b[:], i2b[:], 127.0, None, op0=OP.mult)
            nc.vector.reciprocal_approx_accurate(i2b[:], s2b[:], s1b[:])
            nc.vector.tensor_scalar(i2b[:], i2b[:], invwb[:, 1:2], None, op0=OP.mult)

            # requantize h in place: hq = round(g * s2). The g*s2 mult runs on
            # DVE; the round alternates DVE ts / ScalarE 2-pass (+M then -M)
            # so the engines split the work. Both halves issued before MM2's
            # consumers so Q2(h1) overlaps MM2(h0) matmuls.
            for th in range(2):
                to = th * 512
                for hb in range(HB):
                    gsl = hslice(th, hb)
                    tq = scr2.tile([128, 512], F32, tag="tq")
                    nc.vector.tensor_tensor(tq[:], gsl, s2b[:, to:to + 512], op=OP.mult)
                    if hb % 2 == 0:
                        nc.vector.tensor_scalar(gsl, tq[:], MAGIC, MAGIC,
                                                op0=OP.add, op1=OP.subtract)
                    else:
                        tq2 = scr2.tile([128, 512], F32, tag="tq2")
                        nc.scalar.activation(tq2[:], tq[:], AF.Copy, bias=MAGIC)
                        nc.scalar.activation(gsl, tq2[:], AF.Copy, bias=-MAGIC)

            nr2 = n_cores // 2
            HK = HB // 2
            for th in range(2):
                to = th * 512
                for d in range(DB):
                    wA = w2st.tile([128, HK * 128], F8, tag="wA")
                    nc.sync.dma_start(
                        wA[:].rearrange("k (r f) -> k r f", r=nr2),
                        t2_g[0:nr2, d].rearrange("r k f -> k r f"))
                    wB = w2stB.tile([128, HK * 128], F8, tag="wB")
                    nc.sync.dma_start(
                        wB[:].rearrange("k (r f) -> k r f", r=nr2),
                        t2_g[nr2:n_cores, d].rearrange("r k f -> k r f"))
                    ps = ps_mm.tile([128, 512], F32, tag=f"mm{th}")
                    for kg in range(HB):
                        st, sp = (kg == 0), (kg == HB - 1)
                        w_ = wA if kg < HK else wB
                        ko = (kg % HK) * 128
                        nc.tensor.matmul(ps[:], w_[:, ko:ko + 128],
                                         hslice(th, kg), start=st, stop=sp)
                    ot = hio.tile([128, 512], F32, tag="ot")
                    nc.vector.tensor_tensor(ot[:], ps[:], i2b[:, to:to + 512], op=OP.mult)
                    nc.sync.dma_start(outT[d * 128:(d + 1) * 128, to:to + 512], ot[:])

    nc.compile()
    return nc


def _get_nc():
    if "nc" not in _cache:
        _cache["nc"] = _build()
    return _cache["nc"]


def _prep_inputs(x, w1, w2, gamma):
    x2d = np.ascontiguousarray(np.asarray(x, dtype=np.float32).reshape(NTOK, DIM))
    w1 = np.asarray(w1, dtype=np.float32)
    w2 = np.asarray(w2, dtype=np.float32)
    gamma = np.asarray(gamma, dtype=np.float32)
    w1T = np.ascontiguousarray(w1.T)          # [DIM, HID]
    w2T = np.ascontiguousarray(w2.T)          # [HID, DIM]
    gpt = np.ascontiguousarray(gamma.reshape(KT, 128).T)
    hs = HID // NCORES
    in_maps = []
    for c in range(NCORES):
        in_maps.append({
            "xT": np.ascontiguousarray(x2d[c * TOK:(c + 1) * TOK, :].T),
            "w1s": np.ascontiguousarray(w1T[:, c * hs:(c + 1) * hs]),
            "w2s": np.ascontiguousarray(w2T[c * hs:(c + 1) * hs, :]),
            "gpt": gpt,
        })
    return in_maps


def _run(in_maps, trace=False, **kw):
    nc = _get_nc()
    return bass_utils.run_bass_kernel_spmd(
        nc, in_maps, core_ids=list(range(NCORES)), trace=trace, **kw)


def kernel(x, w1, w2, gamma):
    in_maps = _prep_inputs(x, w1, w2, gamma)
    res = _run(in_maps, trace=False)
    out = np.empty((NTOK, DIM), dtype=np.float32)
    for c in range(NCORES):
        out[c * TOK:(c + 1) * TOK, :] = res.results[c]["outT"].T
    return out.reshape(B, S, DIM)
